# revision 16
# baseline (speedup 1.0000x reference)
"""GATv2 Bass kernel for Trainium2, 8 NeuronCores.

Problem: B=2, N=512, FIN=128, H=4, D=64 GATv2 attention (dense graph).
Sharding: one (batch, head) pair per core (B*H = 8 = n_cores).

Math per (b, h) (reference):
  h[n, d]  = x[n, :] @ W_proj[h]  (+ b_proj[h])
  zi[n, e] = h[n, :] @ W1[h, e, :]      (W1 = W_cat_weight[:, :, :D])
  zj[n, e] = h[n, :] @ W2[h, e, :]      (W2 = W_cat_weight[:, :, D:])
  score[i, j] = sum_e a[h, e] * lrelu(zi[i, e] + zj[j, e] + bcat[h, e])
  attn = softmax_j(score);  out[i, :] = attn[i, :] @ h  (+ bias_param slice)

Kernel decomposition (lrelu(v) = 0.6 v + 0.4 |v|):
  score[i,j] = A_i + B_j + sum_e sign(a_e) * |u[i,j,e]|,
    u = 0.4|a_e| (zi + zj + bcat)   (0.4|a| folded into W1/W2/bcat host-side)
  A_i is constant per row -> drops out of softmax_j. B_j is rank-1, done
  exactly in fp32 via a small matmul, replicated over the i partitions.
  Per core the e-columns are permuted positives-first; the global reduce
  ranges are the sorted unique k+ boundaries across all heads (so the one
  SPMD program fits every core), and each core supplies +-1 range signs
  that ScalarE applies as per-partition scales; GpSimd sums the ranges.
  V[i, (j, e)] = zi''[i, e] + Y''[j, e] is produced by TensorE as ONE
  bf16 matmul per full 512-wide PSUM bank: stationary lhsT = [zi''^T
  rows; zero pad; ones row], moving rhs = [tiled identity; zero pad;
  flattened Y'' row].  VectorE reduces |V| straight out of PSUM with
  tensor_reduce(apply_absolute_value=True) over each range.
"""

import os
import numpy as np
import ml_dtypes

import concourse.bacc as bacc
import concourse.mybir as mybir
import concourse.tile as tile
from concourse.bass_utils import run_bass_kernel_spmd

F32 = mybir.dt.float32
BF16 = mybir.dt.bfloat16
BF = ml_dtypes.bfloat16

B, N, FIN, H, D = 2, 512, 128, 4, 64
NEG_SLOPE = 0.2
C_LIN = (1.0 + NEG_SLOPE) / 2.0   # 0.6
C_ABS = (1.0 - NEG_SLOPE) / 2.0   # 0.4

NB = N // 128             # 4 row blocks of 128
E = D                     # e-dim width (64, unpadded)
J_PB = 512 // E           # j columns per PSUM bank (full 512-elem banks)
N_BANKS = N // J_PB       # banks per i-block
VT_BANKS = 4              # banks per V psum tile
N_VT = N_BANKS // VT_BANKS
VT_J = J_PB * VT_BANKS    # j columns per V tile
IB_N = min(16, N_BANKS)   # identity banks provided via DRAM

last_results = None        # BassKernelResults of the most recent run

_cache = {}


def _build(ranges, use_bcat, use_bproj, use_bias_param):
    """Build + compile the SPMD Bass program. All cores run this one NEFF."""
    nc = bacc.Bacc("TRN2", target_bir_lowering=False, debug=False, num_devices=8)

    nr = len(ranges)
    bounds = np.concatenate([[0], np.cumsum(ranges)]).astype(int)
    assert bounds[-1] == E

    x_d = nc.dram_tensor("x", [N, FIN], F32, kind="ExternalInput")
    wp_d = nc.dram_tensor("wproj", [FIN, D], F32, kind="ExternalInput")
    w1t_d = nc.dram_tensor("w1t", [D, 128], F32, kind="ExternalInput")
    zib_d = nc.dram_tensor("zib", [128, 1], F32, kind="ExternalInput")
    w2t_d = nc.dram_tensor("w2t", [D, E], F32, kind="ExternalInput")
    s15_d = nc.dram_tensor("s15", [E, 128], F32, kind="ExternalInput")
    id16_d = nc.dram_tensor("ident16", [E, IB_N * 512], BF16, kind="ExternalInput")
    id128_d = nc.dram_tensor("id128", [128, 128], F32, kind="ExternalInput")
    id128b_d = nc.dram_tensor("id128b", [128, 128], BF16, kind="ExternalInput")
    grs_d = nc.dram_tensor("grs", [128, nr], F32, kind="ExternalInput")
    if use_bcat:
        bc_d = nc.dram_tensor("bcat", [E, 1], F32, kind="ExternalInput")
    if use_bproj:
        bp_d = nc.dram_tensor("bproj", [D, 1], F32, kind="ExternalInput")
        bpr_d = nc.dram_tensor("bprojrep", [128, D], F32, kind="ExternalInput")
    if use_bias_param:
        bprm_d = nc.dram_tensor("biasprm", [128, D], F32, kind="ExternalInput")
    out_d = nc.dram_tensor("out", [N, D], F32, kind="ExternalOutput")

    AF = mybir.ActivationFunctionType
    ALU = mybir.AluOpType
    AX = mybir.AxisListType

    with tile.TileContext(nc) as tc:
        with tc.tile_pool(name="sb", bufs=1) as sb:
            # ---------- persistent SBUF tiles ----------
            xb = sb.tile([128, NB * 128], F32)
            xT = sb.tile([128, N], F32)
            wp = sb.tile([FIN, D], F32)
            w1t = sb.tile([D, 128], F32)
            zib = sb.tile([128, 1], F32)
            w2t = sb.tile([D, E], F32)
            s15 = sb.tile([E, 128], F32)
            id128 = sb.tile([128, 128], F32)
            id128b = sb.tile([128, 128], BF16)
            grs = sb.tile([128, nr], F32)
            hT = sb.tile([D, N], F32)
            h_sb = sb.tile([128, NB * D], F32)
            ziT = sb.tile([128, N], BF16)             # rows 0:E zi''; 127 ones
            yTf = sb.tile([E, N], F32)
            yTb = sb.tile([E, N], BF16)
            ysb = sb.tile([128, NB * E], BF16)
            rhs_mega = sb.tile([128, N_BANKS * 512], BF16)
            B_sb = sb.tile([128, N], F32)
            R = [[sb.tile([128, N], F32, tag=f"R{i}_{r}", name=f"R{i}_{r}")
                  for r in range(nr)] for i in range(NB)]
            sc = [sb.tile([128, N], F32, tag=f"sc{i}", name=f"sc{i}")
                  for i in range(NB)]
            tg = [sb.tile([128, N], F32, tag=f"tg{i}", name=f"tg{i}")
                  for i in range(NB)]
            ee = [sb.tile([128, N], F32, tag=f"ee{i}", name=f"ee{i}")
                  for i in range(NB)]
            rowmax = [sb.tile([128, 1], F32, tag=f"rm{i}", name=f"rm{i}")
                      for i in range(NB)]
            negm = [sb.tile([128, 1], F32, tag=f"nm{i}", name=f"nm{i}")
                    for i in range(NB)]
            zsum = [sb.tile([128, 1], F32, tag=f"zs{i}", name=f"zs{i}")
                    for i in range(NB)]
            rz = [sb.tile([128, 1], F32, tag=f"rz{i}", name=f"rz{i}")
                  for i in range(NB)]
            if use_bcat:
                bc = sb.tile([E, 1], F32)
            if use_bproj:
                bp = sb.tile([D, 1], F32)
                bpr = sb.tile([128, D], F32)
            if use_bias_param:
                bprm = sb.tile([128, D], F32)

            # ---------- input DMAs ----------
            nc.sync.dma_start(
                xb[:, :].rearrange("p (nb f) -> p nb f", nb=NB),
                x_d.ap().rearrange("(nb p) f -> p nb f", p=128))
            nc.sync.dma_start(wp[:], wp_d.ap())
            nc.sync.dma_start(w1t[:], w1t_d.ap())
            nc.sync.dma_start(zib[:], zib_d.ap())
            nc.sync.dma_start(w2t[:], w2t_d.ap())
            nc.sync.dma_start(s15[:], s15_d.ap())
            nc.sync.dma_start(id128[:], id128_d.ap())
            nc.sync.dma_start(id128b[:], id128b_d.ap())
            nc.sync.dma_start(grs[:], grs_d.ap())
            if use_bcat:
                nc.sync.dma_start(bc[:], bc_d.ap())
            if use_bproj:
                nc.sync.dma_start(bp[:], bp_d.ap())
                nc.sync.dma_start(bpr[:], bpr_d.ap())
            if use_bias_param:
                nc.sync.dma_start(bprm[:], bprm_d.ap())

            # rhs_mega: zero pad rows + identity doubling on ScalarE (off DVE)
            nc.scalar.memzero(rhs_mega[64:128, 0:IB_N * 512])
            nc.sync.dma_start(rhs_mega[0:E, 0:IB_N * 512], id16_d.ap())
            m = IB_N
            while m < N_BANKS:
                c = min(m, N_BANKS - m)
                nc.scalar.copy(
                    rhs_mega[0:127, m * 512:(m + c) * 512],
                    rhs_mega[0:127, 0:c * 512])
                m += c

            # ---------- prep phase ----------
            with tc.tile_pool(name="pp", bufs=4, space="PSUM") as pp:
                for nb in range(NB):
                    t = pp.tile([128, 512], F32, tag="t")
                    nc.tensor.transpose(t[:, 0:128], xb[:, nb * 128:(nb + 1) * 128],
                                        id128[:])
                    nc.scalar.copy(xT[:, nb * 128:(nb + 1) * 128], t[:, 0:128])
                t = pp.tile([D, N], F32, tag="t")
                nc.tensor.matmul(t[:], wp[:], xT[:])
                if use_bproj:
                    nc.scalar.activation(hT[:], t[:], AF.Identity, bias=bp[:, 0:1])
                else:
                    nc.scalar.copy(hT[:], t[:])
                for nb in range(NB):
                    t = pp.tile([128, 512], F32, tag="t")
                    nc.tensor.matmul(t[:, 0:D], xT[:, nb * 128:(nb + 1) * 128], wp[:])
                    if use_bproj:
                        nc.vector.tensor_tensor(
                            h_sb[:, nb * D:(nb + 1) * D], t[:, 0:D], bpr[:],
                            op=ALU.add)
                    else:
                        nc.scalar.copy(h_sb[:, nb * D:(nb + 1) * D], t[:, 0:D])
                # zi''^T (row 127 -> 1.0 via bias; rows E..126 -> 0 via w1t pad)
                t = pp.tile([128, N], F32, tag="t")
                nc.tensor.matmul(t[:], w1t[:], hT[:])
                nc.scalar.activation(ziT[:], t[:], AF.Identity, bias=zib[:, 0:1])
                # Y''^T
                t = pp.tile([E, N], F32, tag="t")
                nc.tensor.matmul(t[:], w2t[:], hT[:])
                if use_bcat:
                    nc.scalar.activation(yTf[:], t[:], AF.Identity, bias=bc[:, 0:1])
                    nc.vector.tensor_copy(yTb[:], yTf[:])
                else:
                    nc.scalar.copy(yTf[:], t[:])
                    nc.vector.tensor_copy(yTb[:], t[:])
                # B_j replicated over i
                t = pp.tile([128, N], F32, tag="t")
                nc.tensor.matmul(t[:], s15[:], yTf[:])
                nc.scalar.copy(B_sb[:], t[:])
                # Y'' row blocks (n x e)
                for nb in range(NB):
                    t = pp.tile([128, 512], BF16, tag="tb")
                    nc.tensor.transpose(
                        t[:, 0:E], yTb[:, nb * 128:(nb + 1) * 128],
                        id128b[0:E, 0:E])
                    nc.scalar.copy(ysb[:, nb * E:(nb + 1) * E], t[:, 0:E])
                # flatten Y'' into rhs_mega row 127
                for nb in range(NB):
                    dst = rhs_mega[127:128, :].rearrange("o (n e) -> o n e", e=E)
                    nc.sync.dma_start(
                        dst[:, nb * 128:(nb + 1) * 128, :],
                        ysb[:, nb * E:(nb + 1) * E])

            # ---------- main loop ----------
            with tc.tile_pool(name="vp", bufs=2, space="PSUM") as vp:
                for ib in range(NB):
                    zi_l = ziT[:, ib * 128:(ib + 1) * 128]
                    for vt in range(N_VT):
                        v = vp.tile([128, VT_BANKS * 512], F32, tag="v")
                        for k in range(VT_BANKS):
                            bb = vt * VT_BANKS + k
                            nc.tensor.matmul(
                                v[:, k * 512:(k + 1) * 512], zi_l,
                                rhs_mega[:, bb * 512:(bb + 1) * 512],
                                start=True, stop=True)
                        v4 = v[:, :].rearrange("p (j e) -> p j e", e=E)
                        for r in range(nr):
                            ro = R[ib][r][:, vt * VT_J:(vt + 1) * VT_J]
                            nc.vector.tensor_reduce(
                                ro, v4[:, :, int(bounds[r]):int(bounds[r + 1])],
                                axis=AX.X, op=ALU.add,
                                apply_absolute_value=True)
                    # combine: sc = sum_r g_r * R_r + B (ScalarE mults, GpSimd adds)
                    nc.scalar.activation(tg[ib][:], R[ib][0][:], AF.Copy, bias=0.0,
                                         scale=grs[:, 0:1])
                    for r in range(1, nr):
                        nc.scalar.activation(R[ib][r][:], R[ib][r][:], AF.Copy,
                                             bias=0.0, scale=grs[:, r:r + 1])
                    first = R[ib][1][:] if nr > 1 else B_sb[:]
                    nc.gpsimd.tensor_tensor(sc[ib][:], tg[ib][:], first, op=ALU.add)
                    for r in range(2, nr):
                        nc.gpsimd.tensor_tensor(sc[ib][:], sc[ib][:], R[ib][r][:],
                                                op=ALU.add)
                    if nr > 1:
                        nc.gpsimd.tensor_tensor(sc[ib][:], sc[ib][:], B_sb[:],
                                                op=ALU.add)
                    # softmax pieces
                    nc.vector.tensor_reduce(rowmax[ib][:], sc[ib][:],
                                            axis=AX.X, op=ALU.max)
                    nc.vector.tensor_scalar_mul(negm[ib][:], rowmax[ib][:], -1.0)
                    nc.scalar.activation(ee[ib][:], sc[ib][:], AF.Exp,
                                         bias=negm[ib][:, 0:1],
                                         accum_out=zsum[ib][:])
                    nc.vector.reciprocal(rz[ib][:], zsum[ib][:])

            # ---------- epilogue: attn @ h ----------
            with tc.tile_pool(name="ep", bufs=2, space="PSUM") as ep:
                for ib in range(NB):
                    eT = sb.tile([128, 128 * NB], F32, tag=f"eT{ib}",
                                 name=f"eT{ib}")
                    for jb in range(NB):
                        t = ep.tile([128, 128], F32, tag="et")
                        nc.tensor.transpose(
                            t[:], ee[ib][:, jb * 128:(jb + 1) * 128], id128[:])
                        nc.scalar.copy(eT[:, jb * 128:(jb + 1) * 128], t[:])
                    acc = ep.tile([128, D], F32, tag="acc")
                    for jb in range(NB):
                        nc.tensor.matmul(
                            acc[:], eT[:, jb * 128:(jb + 1) * 128],
                            h_sb[:, jb * D:(jb + 1) * D],
                            start=(jb == 0), stop=(jb == NB - 1))
                    o = sb.tile([128, D], F32, tag=f"o{ib}", name=f"o{ib}")
                    nc.scalar.activation(o[:], acc[:], AF.Copy, bias=0.0,
                                         scale=rz[ib][:, 0:1])
                    if use_bias_param:
                        nc.vector.tensor_tensor(o[:], o[:], bprm[:], op=ALU.add)
                    nc.sync.dma_start(out_d.ap()[ib * 128:(ib + 1) * 128, :], o[:])

    nc.compile()
    return nc


def kernel(x, W_proj, b_proj, W_cat_weight, W_cat_bias, a, bias_param):
    global last_results
    x = np.asarray(x, dtype=np.float32)
    W_proj = np.asarray(W_proj, dtype=np.float32)
    b_proj = np.asarray(b_proj, dtype=np.float32)
    W_cat_weight = np.asarray(W_cat_weight, dtype=np.float32)
    W_cat_bias = np.asarray(W_cat_bias, dtype=np.float32)
    a = np.asarray(a, dtype=np.float32)
    bias_param = np.asarray(bias_param, dtype=np.float32)

    W1 = W_cat_weight[:, :, :D]
    W2 = W_cat_weight[:, :, D:]

    kpos = [int((a[h] > 0).sum()) for h in range(H)]
    pts = sorted({k for k in kpos if 0 < k < E})
    widths = tuple(int(w) for w in np.diff([0] + pts + [E]) if w > 0) or (E,)

    use_bcat = bool(np.any(W_cat_bias))
    use_bproj = bool(np.any(b_proj))
    use_bias_param = bool(np.any(bias_param))

    key = (widths, use_bcat, use_bproj, use_bias_param)
    if key not in _cache:
        _cache[key] = _build(*key)
    nc = _cache[key]

    nr = len(widths)
    bounds = np.concatenate([[0], np.cumsum(widths)]).astype(int)
    ident16 = np.tile(np.eye(E, dtype=np.float32), (1, IB_N * J_PB)).astype(BF)
    id128 = np.eye(128, dtype=np.float32)

    in_maps = []
    for c in range(8):
        b, h = divmod(c, H)
        ah = a[h]
        pos = np.where(ah > 0)[0]
        neg = np.where(ah <= 0)[0]
        kp = len(pos)
        slots = np.concatenate([pos, neg])
        scale = C_ABS * np.abs(ah[slots])        # 0.4|a| per slot
        sgn = np.sign(ah[slots])
        W1p = np.zeros((128, D), dtype=np.float32)
        W2p = np.zeros((E, D), dtype=np.float32)
        W1p[:E] = W1[h][slots] * scale[:, None]
        W2p[:] = W2[h][slots] * scale[:, None]
        s15 = np.tile((1.5 * sgn)[:, None], (1, 128)).astype(np.float32)
        # range r is all-positive for this core iff its end <= kp (or kp == E)
        g = np.array([1.0 if (bounds[r + 1] <= kp or kp == E) else -1.0
                      for r in range(nr)], dtype=np.float32)
        grs = np.tile(g[None, :], (128, 1)).astype(np.float32)
        m = {
            "x": np.ascontiguousarray(x[b]),
            "wproj": np.ascontiguousarray(W_proj[h]),
            "w1t": np.ascontiguousarray(W1p.T),
            "zib": np.eye(1, 128, 127, dtype=np.float32).T.copy(),
            "w2t": np.ascontiguousarray(W2p.T),
            "s15": s15,
            "ident16": ident16,
            "id128": id128,
            "id128b": id128.astype(BF),
            "grs": grs,
        }
        if use_bcat:
            m["bcat"] = (W_cat_bias[h][slots] * scale)[:, None].astype(np.float32)
        if use_bproj:
            m["bproj"] = np.ascontiguousarray(b_proj[h][:, None])
            m["bprojrep"] = np.tile(b_proj[h][None, :], (128, 1)).astype(np.float32)
        if use_bias_param:
            m["biasprm"] = np.tile(bias_param[None, h * D:(h + 1) * D],
                                   (128, 1)).astype(np.float32)
        in_maps.append(m)

    res = run_bass_kernel_spmd(nc, in_maps, core_ids=list(range(8)))
    last_results = res

    out = np.empty((B, N, H * D), dtype=np.float32)
    for c in range(8):
        b, h = divmod(c, H)
        out[b, :, h * D:(h + 1) * D] = res.results[c]["out"]
    return out


# revision 17
# speedup vs baseline: 1.0671x; 1.0671x over previous
"""GATv2 Bass kernel for Trainium2, 8 NeuronCores.

Problem: B=2, N=512, FIN=128, H=4, D=64 GATv2 attention (dense graph).
Sharding: one (batch, head) pair per core (B*H = 8 = n_cores).

Math per (b, h) (reference):
  h[n, d]  = x[n, :] @ W_proj[h]  (+ b_proj[h])
  zi[n, e] = h[n, :] @ W1[h, e, :]      (W1 = W_cat_weight[:, :, :D])
  zj[n, e] = h[n, :] @ W2[h, e, :]      (W2 = W_cat_weight[:, :, D:])
  score[i, j] = sum_e a[h, e] * lrelu(zi[i, e] + zj[j, e] + bcat[h, e])
  attn = softmax_j(score);  out[i, :] = attn[i, :] @ h  (+ bias_param slice)

Kernel decomposition (lrelu(v) = 0.6 v + 0.4 |v|):
  score[i,j] = A_i + B_j + sum_e sign(a_e) * |u[i,j,e]|,
    u = 0.4|a_e| (zi + zj + bcat)   (0.4|a| folded into W1/W2/bcat host-side)
  A_i is constant per row -> drops out of softmax_j. B_j is rank-1, done
  exactly in fp32 via a small matmul, replicated over the i partitions.
  Per core the e-columns are permuted positives-first; the global reduce
  ranges are the sorted unique k+ boundaries across all heads (so the one
  SPMD program fits every core), and each core supplies +-1 range signs
  that ScalarE applies as per-partition scales; GpSimd sums the ranges.
  V[i, (j, e)] = zi''[i, e] + Y''[j, e] is produced by TensorE as ONE
  bf16 matmul per full 512-wide PSUM bank: stationary lhsT = [zi''^T
  rows; zero pad; ones row], moving rhs = [tiled identity; zero pad;
  flattened Y'' row].  VectorE reduces |V| straight out of PSUM with
  tensor_reduce(apply_absolute_value=True) over each range.
"""

import os
import numpy as np
import ml_dtypes

import concourse.bacc as bacc
import concourse.mybir as mybir
import concourse.tile as tile
from concourse.bass_utils import run_bass_kernel_spmd

F32 = mybir.dt.float32
BF16 = mybir.dt.bfloat16
BF = ml_dtypes.bfloat16

B, N, FIN, H, D = 2, 512, 128, 4, 64
NEG_SLOPE = 0.2
C_LIN = (1.0 + NEG_SLOPE) / 2.0   # 0.6
C_ABS = (1.0 - NEG_SLOPE) / 2.0   # 0.4

NB = N // 128             # 4 row blocks of 128
E = D                     # e-dim width (64, unpadded)
J_PB = 512 // E           # j columns per PSUM bank (full 512-elem banks)
N_BANKS = N // J_PB       # banks per i-block
VT_BANKS = 4              # banks per V psum tile
N_VT = N_BANKS // VT_BANKS
VT_J = J_PB * VT_BANKS    # j columns per V tile
IB_N = min(16, N_BANKS)   # identity banks provided via DRAM

last_results = None        # BassKernelResults of the most recent run

_cache = {}


def _build(ranges, use_bcat, use_bproj, use_bias_param):
    """Build + compile the SPMD Bass program. All cores run this one NEFF."""
    nc = bacc.Bacc("TRN2", target_bir_lowering=False, debug=False, num_devices=8)

    nr = len(ranges)
    bounds = np.concatenate([[0], np.cumsum(ranges)]).astype(int)
    assert bounds[-1] == E

    x_d = nc.dram_tensor("x", [N, FIN], F32, kind="ExternalInput")
    wp_d = nc.dram_tensor("wproj", [FIN, D], F32, kind="ExternalInput")
    w1t_d = nc.dram_tensor("w1t", [D, 128], F32, kind="ExternalInput")
    zib_d = nc.dram_tensor("zib", [128, 1], F32, kind="ExternalInput")
    w2t_d = nc.dram_tensor("w2t", [D, E], F32, kind="ExternalInput")
    s15_d = nc.dram_tensor("s15", [E, 128], F32, kind="ExternalInput")
    id16_d = nc.dram_tensor("ident16", [E, IB_N * 512], BF16, kind="ExternalInput")
    id128_d = nc.dram_tensor("id128", [128, 128], F32, kind="ExternalInput")
    id128b_d = nc.dram_tensor("id128b", [128, 128], BF16, kind="ExternalInput")
    grs_d = nc.dram_tensor("grs", [128, nr], F32, kind="ExternalInput")
    if use_bcat:
        bc_d = nc.dram_tensor("bcat", [E, 1], F32, kind="ExternalInput")
    if use_bproj:
        bp_d = nc.dram_tensor("bproj", [D, 1], F32, kind="ExternalInput")
        bpr_d = nc.dram_tensor("bprojrep", [128, D], F32, kind="ExternalInput")
    if use_bias_param:
        bprm_d = nc.dram_tensor("biasprm", [128, D], F32, kind="ExternalInput")
    out_d = nc.dram_tensor("out", [N, D], F32, kind="ExternalOutput")

    AF = mybir.ActivationFunctionType
    ALU = mybir.AluOpType
    AX = mybir.AxisListType

    with tile.TileContext(nc) as tc:
        with tc.tile_pool(name="sb", bufs=1) as sb:
            # ---------- persistent SBUF tiles ----------
            xb = sb.tile([128, NB * 128], F32)
            xT = sb.tile([128, N], F32)
            wp = sb.tile([FIN, D], F32)
            w1t = sb.tile([D, 128], F32)
            zib = sb.tile([128, 1], F32)
            w2t = sb.tile([D, E], F32)
            s15 = sb.tile([E, 128], F32)
            id128 = sb.tile([128, 128], F32)
            id128b = sb.tile([128, 128], BF16)
            grs = sb.tile([128, nr], F32)
            hT = sb.tile([D, N], F32)
            h_sb = sb.tile([128, NB * D], F32)
            ziT = sb.tile([128, N], BF16)             # rows 0:E zi''; 127 ones
            yTf = sb.tile([E, N], F32)
            yTb = sb.tile([E, N], BF16)
            ysb = sb.tile([128, NB * E], BF16)
            rhs_mega = sb.tile([128, N_BANKS * 512], BF16)
            B_sb = sb.tile([128, N], F32)
            R = [[sb.tile([128, N], F32, tag=f"R{i}_{r}", name=f"R{i}_{r}")
                  for r in range(nr)] for i in range(NB)]
            sc = [sb.tile([128, N], F32, tag=f"sc{i}", name=f"sc{i}")
                  for i in range(NB)]
            tg = [sb.tile([128, N], F32, tag=f"tg{i}", name=f"tg{i}")
                  for i in range(NB)]
            ee = [sb.tile([128, N], F32, tag=f"ee{i}", name=f"ee{i}")
                  for i in range(NB)]
            rowmax = [sb.tile([128, 1], F32, tag=f"rm{i}", name=f"rm{i}")
                      for i in range(NB)]
            negm = [sb.tile([128, 1], F32, tag=f"nm{i}", name=f"nm{i}")
                    for i in range(NB)]
            zsum = [sb.tile([128, 1], F32, tag=f"zs{i}", name=f"zs{i}")
                    for i in range(NB)]
            rz = [sb.tile([128, 1], F32, tag=f"rz{i}", name=f"rz{i}")
                  for i in range(NB)]
            if use_bcat:
                bc = sb.tile([E, 1], F32)
            if use_bproj:
                bp = sb.tile([D, 1], F32)
                bpr = sb.tile([128, D], F32)
            if use_bias_param:
                bprm = sb.tile([128, D], F32)

            # ---------- input DMAs ----------
            nc.sync.dma_start(
                xb[:, :].rearrange("p (nb f) -> p nb f", nb=NB),
                x_d.ap().rearrange("(nb p) f -> p nb f", p=128))
            nc.sync.dma_start(wp[:], wp_d.ap())
            nc.sync.dma_start(w1t[:], w1t_d.ap())
            nc.sync.dma_start(zib[:], zib_d.ap())
            nc.sync.dma_start(w2t[:], w2t_d.ap())
            nc.sync.dma_start(s15[:], s15_d.ap())
            nc.sync.dma_start(id128[:], id128_d.ap())
            nc.sync.dma_start(id128b[:], id128b_d.ap())
            nc.sync.dma_start(grs[:], grs_d.ap())
            if use_bcat:
                nc.sync.dma_start(bc[:], bc_d.ap())
            if use_bproj:
                nc.sync.dma_start(bp[:], bp_d.ap())
                nc.sync.dma_start(bpr[:], bpr_d.ap())
            if use_bias_param:
                nc.sync.dma_start(bprm[:], bprm_d.ap())

            # rhs_mega: zero pad rows + identity doubling on ScalarE (off DVE)
            nc.scalar.memzero(rhs_mega[64:128, 0:IB_N * 512])
            nc.sync.dma_start(rhs_mega[0:E, 0:IB_N * 512], id16_d.ap())
            m = IB_N
            while m < N_BANKS:
                c = min(m, N_BANKS - m)
                nc.vector.tensor_copy(
                    rhs_mega[0:127, m * 512:(m + c) * 512],
                    rhs_mega[0:127, 0:c * 512])
                m += c

            # ---------- prep phase ----------
            with tc.tile_pool(name="pp", bufs=4, space="PSUM") as pp:
                for nb in range(NB):
                    t = pp.tile([128, 512], F32, tag="t")
                    nc.tensor.transpose(t[:, 0:128], xb[:, nb * 128:(nb + 1) * 128],
                                        id128[:])
                    nc.scalar.copy(xT[:, nb * 128:(nb + 1) * 128], t[:, 0:128])
                t = pp.tile([D, N], F32, tag="t")
                nc.tensor.matmul(t[:], wp[:], xT[:])
                if use_bproj:
                    nc.scalar.activation(hT[:], t[:], AF.Identity, bias=bp[:, 0:1])
                else:
                    nc.scalar.copy(hT[:], t[:])
                for nb in range(NB):
                    t = pp.tile([128, 512], F32, tag="t")
                    nc.tensor.matmul(t[:, 0:D], xT[:, nb * 128:(nb + 1) * 128], wp[:])
                    if use_bproj:
                        nc.vector.tensor_tensor(
                            h_sb[:, nb * D:(nb + 1) * D], t[:, 0:D], bpr[:],
                            op=ALU.add)
                    else:
                        nc.scalar.copy(h_sb[:, nb * D:(nb + 1) * D], t[:, 0:D])
                # zi''^T (row 127 -> 1.0 via bias; rows E..126 -> 0 via w1t pad)
                t = pp.tile([128, N], F32, tag="t")
                nc.tensor.matmul(t[:], w1t[:], hT[:])
                nc.scalar.activation(ziT[:], t[:], AF.Identity, bias=zib[:, 0:1])
                # Y''^T
                t = pp.tile([E, N], F32, tag="t")
                nc.tensor.matmul(t[:], w2t[:], hT[:])
                if use_bcat:
                    nc.scalar.activation(yTf[:], t[:], AF.Identity, bias=bc[:, 0:1])
                    nc.vector.tensor_copy(yTb[:], yTf[:])
                else:
                    nc.scalar.copy(yTf[:], t[:])
                    nc.vector.tensor_copy(yTb[:], t[:])
                # B_j replicated over i
                t = pp.tile([128, N], F32, tag="t")
                nc.tensor.matmul(t[:], s15[:], yTf[:])
                nc.scalar.copy(B_sb[:], t[:])
                # Y'' row blocks (n x e)
                for nb in range(NB):
                    t = pp.tile([128, 512], BF16, tag="tb")
                    nc.tensor.transpose(
                        t[:, 0:E], yTb[:, nb * 128:(nb + 1) * 128],
                        id128b[0:E, 0:E])
                    nc.scalar.copy(ysb[:, nb * E:(nb + 1) * E], t[:, 0:E])
                # flatten Y'' into rhs_mega row 127
                for nb in range(NB):
                    dst = rhs_mega[127:128, :].rearrange("o (n e) -> o n e", e=E)
                    nc.sync.dma_start(
                        dst[:, nb * 128:(nb + 1) * 128, :],
                        ysb[:, nb * E:(nb + 1) * E])

            # ---------- main loop ----------
            with tc.tile_pool(name="vp", bufs=2, space="PSUM") as vp:
                for ib in range(NB):
                    zi_l = ziT[:, ib * 128:(ib + 1) * 128]
                    for vt in range(N_VT):
                        v = vp.tile([128, VT_BANKS * 512], F32, tag="v")
                        for k in range(VT_BANKS):
                            bb = vt * VT_BANKS + k
                            nc.tensor.matmul(
                                v[:, k * 512:(k + 1) * 512], zi_l,
                                rhs_mega[:, bb * 512:(bb + 1) * 512],
                                start=True, stop=True)
                        v4 = v[:, :].rearrange("p (j e) -> p j e", e=E)
                        for r in range(nr):
                            ro = R[ib][r][:, vt * VT_J:(vt + 1) * VT_J]
                            nc.vector.tensor_reduce(
                                ro, v4[:, :, int(bounds[r]):int(bounds[r + 1])],
                                axis=AX.X, op=ALU.add,
                                apply_absolute_value=True)
                    # combine: sc = sum_r g_r * R_r + B (ScalarE mults, GpSimd adds)
                    nc.scalar.activation(tg[ib][:], R[ib][0][:], AF.Copy, bias=0.0,
                                         scale=grs[:, 0:1])
                    for r in range(1, nr):
                        nc.scalar.activation(R[ib][r][:], R[ib][r][:], AF.Copy,
                                             bias=0.0, scale=grs[:, r:r + 1])
                    first = R[ib][1][:] if nr > 1 else B_sb[:]
                    nc.gpsimd.tensor_tensor(sc[ib][:], tg[ib][:], first, op=ALU.add)
                    for r in range(2, nr):
                        nc.gpsimd.tensor_tensor(sc[ib][:], sc[ib][:], R[ib][r][:],
                                                op=ALU.add)
                    if nr > 1:
                        nc.gpsimd.tensor_tensor(sc[ib][:], sc[ib][:], B_sb[:],
                                                op=ALU.add)
                    # softmax pieces
                    nc.vector.tensor_reduce(rowmax[ib][:], sc[ib][:],
                                            axis=AX.X, op=ALU.max)
                    nc.vector.tensor_scalar_mul(negm[ib][:], rowmax[ib][:], -1.0)
                    nc.scalar.activation(ee[ib][:], sc[ib][:], AF.Exp,
                                         bias=negm[ib][:, 0:1],
                                         accum_out=zsum[ib][:])
                    nc.vector.reciprocal(rz[ib][:], zsum[ib][:])

            # ---------- epilogue: attn @ h ----------
            with tc.tile_pool(name="ep", bufs=2, space="PSUM") as ep:
                for ib in range(NB):
                    eT = sb.tile([128, 128 * NB], F32, tag=f"eT{ib}",
                                 name=f"eT{ib}")
                    for jb in range(NB):
                        t = ep.tile([128, 128], F32, tag="et")
                        nc.tensor.transpose(
                            t[:], ee[ib][:, jb * 128:(jb + 1) * 128], id128[:])
                        nc.scalar.copy(eT[:, jb * 128:(jb + 1) * 128], t[:])
                    acc = ep.tile([128, D], F32, tag="acc")
                    for jb in range(NB):
                        nc.tensor.matmul(
                            acc[:], eT[:, jb * 128:(jb + 1) * 128],
                            h_sb[:, jb * D:(jb + 1) * D],
                            start=(jb == 0), stop=(jb == NB - 1))
                    o = sb.tile([128, D], F32, tag=f"o{ib}", name=f"o{ib}")
                    nc.scalar.activation(o[:], acc[:], AF.Copy, bias=0.0,
                                         scale=rz[ib][:, 0:1])
                    if use_bias_param:
                        nc.vector.tensor_tensor(o[:], o[:], bprm[:], op=ALU.add)
                    nc.sync.dma_start(out_d.ap()[ib * 128:(ib + 1) * 128, :], o[:])

    nc.compile()
    return nc


def kernel(x, W_proj, b_proj, W_cat_weight, W_cat_bias, a, bias_param):
    global last_results
    x = np.asarray(x, dtype=np.float32)
    W_proj = np.asarray(W_proj, dtype=np.float32)
    b_proj = np.asarray(b_proj, dtype=np.float32)
    W_cat_weight = np.asarray(W_cat_weight, dtype=np.float32)
    W_cat_bias = np.asarray(W_cat_bias, dtype=np.float32)
    a = np.asarray(a, dtype=np.float32)
    bias_param = np.asarray(bias_param, dtype=np.float32)

    W1 = W_cat_weight[:, :, :D]
    W2 = W_cat_weight[:, :, D:]

    kpos = [int((a[h] > 0).sum()) for h in range(H)]
    pts = sorted({k for k in kpos if 0 < k < E})
    widths = tuple(int(w) for w in np.diff([0] + pts + [E]) if w > 0) or (E,)

    use_bcat = bool(np.any(W_cat_bias))
    use_bproj = bool(np.any(b_proj))
    use_bias_param = bool(np.any(bias_param))

    key = (widths, use_bcat, use_bproj, use_bias_param)
    if key not in _cache:
        _cache[key] = _build(*key)
    nc = _cache[key]

    nr = len(widths)
    bounds = np.concatenate([[0], np.cumsum(widths)]).astype(int)
    ident16 = np.tile(np.eye(E, dtype=np.float32), (1, IB_N * J_PB)).astype(BF)
    id128 = np.eye(128, dtype=np.float32)

    in_maps = []
    for c in range(8):
        b, h = divmod(c, H)
        ah = a[h]
        pos = np.where(ah > 0)[0]
        neg = np.where(ah <= 0)[0]
        kp = len(pos)
        slots = np.concatenate([pos, neg])
        scale = C_ABS * np.abs(ah[slots])        # 0.4|a| per slot
        sgn = np.sign(ah[slots])
        W1p = np.zeros((128, D), dtype=np.float32)
        W2p = np.zeros((E, D), dtype=np.float32)
        W1p[:E] = W1[h][slots] * scale[:, None]
        W2p[:] = W2[h][slots] * scale[:, None]
        s15 = np.tile((1.5 * sgn)[:, None], (1, 128)).astype(np.float32)
        # range r is all-positive for this core iff its end <= kp (or kp == E)
        g = np.array([1.0 if (bounds[r + 1] <= kp or kp == E) else -1.0
                      for r in range(nr)], dtype=np.float32)
        grs = np.tile(g[None, :], (128, 1)).astype(np.float32)
        m = {
            "x": np.ascontiguousarray(x[b]),
            "wproj": np.ascontiguousarray(W_proj[h]),
            "w1t": np.ascontiguousarray(W1p.T),
            "zib": np.eye(1, 128, 127, dtype=np.float32).T.copy(),
            "w2t": np.ascontiguousarray(W2p.T),
            "s15": s15,
            "ident16": ident16,
            "id128": id128,
            "id128b": id128.astype(BF),
            "grs": grs,
        }
        if use_bcat:
            m["bcat"] = (W_cat_bias[h][slots] * scale)[:, None].astype(np.float32)
        if use_bproj:
            m["bproj"] = np.ascontiguousarray(b_proj[h][:, None])
            m["bprojrep"] = np.tile(b_proj[h][None, :], (128, 1)).astype(np.float32)
        if use_bias_param:
            m["biasprm"] = np.tile(bias_param[None, h * D:(h + 1) * D],
                                   (128, 1)).astype(np.float32)
        in_maps.append(m)

    res = run_bass_kernel_spmd(nc, in_maps, core_ids=list(range(8)))
    last_results = res

    out = np.empty((B, N, H * D), dtype=np.float32)
    for c in range(8):
        b, h = divmod(c, H)
        out[b, :, h * D:(h + 1) * D] = res.results[c]["out"]
    return out
